# revision 81
# baseline (speedup 1.0000x reference)
"""Causal depthwise conv1d with learnable hidden-state prefix, on 8 TRN2 cores.

Reference computation (per batch b, channel d):
    xp = concat([init_state[d, :3], x[b, d, :]])          # [L+3] = [4099]
    out[b, d, t] = bias[d] + sum_{j=0..3} w[d, j] * xp[t+j]   for t in [0, 4099)
    (xp index beyond 4098 contributes 0)

Sharding: channel dim D=4096 split 8 ways (512 channels/core), zero
communication. Each core processes rows (b, d_local) = 4*512 = 2048 rows of
length 4096 -> 16 SBUF tiles of [128 rows, full row].

I/O strategy: x streams in as fp32 (exact), the result streams out as fp16
(one final rounding of each output value: rel err ~2^-11, far inside the
tolerance) and is upcast to fp32 on the host. That puts the per-core DMA
floor at 32MB in + 16MB out ~= 140us @ 360GB/s.

Compute: all engines share the row so each stays under the DMA floor. Per
tile the columns split three ways:
  - PE chunks (exact fp32 matmuls, diagonal weight per tap, 4 taps
    accumulated in PSUM); ACT evacuates + bias -> fp16.  (float32r would be
    4x cheaper but is bf16-rounded on hardware -- fails the small-|y|
    relative-error floor.)
  - DVE region: ACT does tap0+bias into an fp32 scratch, DVE runs fused
    scalar*tensor+tensor MACs for taps 1-2, tap 3 writes fp16 directly.
  - Pool region: GPSIMD can't run scalar_tensor_tensor (walrus ISA check),
    so its tap0 is a DVE tensor_scalar (2x mode for fp32), ACT forms each
    tap product via its free per-partition scale (tmp = w_j * x_shift),
    and Pool does plain tensor_tensor adds; the last add writes fp16.

Scheduling: tile_wait_until clock-waits hold each tile's out-DMA to
~(25 + 5t) us and pace the matmuls to ~(25 + 6.5t) us, so the exclusive
DMA engines stream all in-DMAs back-to-back first and the held outs fill
the drain tail; the last two tiles' outs split at col 1024 and lean on
the PE (it drains ~15us before the other engines).

Engine budget per core (TimelineSim): DMA 140.1us (the floor), PE ~122,
Pool ~120, DVE ~115, ACT ~112 -> 145,121 ns vs the 139,833 ns pure-DMA
bound; rel err 5.0e-4 on device (gate 2e-2).
"""

import numpy as np

B, D, L = 4, 4096, 4096
KTAPS = 4
K = KTAPS - 1          # 3: state length
LOUT = L + K           # 4099
NCORES = 8
DSH = D // NCORES      # 512 channels per core
ROWS = B * DSH         # 2048 rows per core
P = 128                # SBUF partitions
NTILES = ROWS // P     # 16
G = DSH // P           # 4 channel groups per core

_CACHE = {}

MMCOLS = 512           # one PSUM bank of fp32 per matmul
# fp32 matmul chunks per tile; the last two tiles lean on the PE (it drains
# ~15us before the other engines, and a bigger PE share shortens the final
# MAC chains that gate the drain tail)
PE_CHUNKS = (2,) * (NTILES - 2) + (3, 3)
DVE_COLS = 2048            # DVE-region width; Pool gets the remainder


def _build_program(pe_chunks=PE_CHUNKS, dve_cols=DVE_COLS, in_bufs=6,
                   out_bufs=5, scr_bufs=2, tmp_bufs=2, feed='act',
                   split_in=(), chain_splits=1, col_pieces=1, lt_pe=None,
                   lt_dve=None, out_cols=(1024,), tap0_d='act',
                   tap0_p='dve', evac='act', p_tap0_first=True,
                   out_split_tiles=2, out_lag=0, out_wait=(25, 5.0),
                   pe_wait=(25, 6.5), tap0_d_hi=False):
    import concourse.bacc as bacc
    import concourse.mybir as mybir
    from concourse.tile import TileContext

    f32 = mybir.dt.float32
    f16 = mybir.dt.float16
    nc = bacc.Bacc("TRN2", target_bir_lowering=False, debug=False)

    xs = nc.dram_tensor("xs", [ROWS, L], f32, kind="ExternalInput").ap()
    # single packed param tensor -> single DMA -> single sync wait downstream.
    # layout per partition p: cols [g*4+j]=w[g*128+p, j] for g<4,j<4 (0..16),
    # col 16+g = bias[g*128+p], col 20+g*3+k = init_state[g*128+p, k]
    # prm and eye ride ONE dram tensor/DMA: a second HWDGE generation on
    # the SP ring would delay the first x tile's transfer by ~625ns
    cst_d = nc.dram_tensor("cst", [P, 32 + P], f32, kind="ExternalInput").ap()
    out_d = nc.dram_tensor("out", [ROWS, LOUT], f16, kind="ExternalOutput").ap()

    with TileContext(nc) as tc:
        with (
            tc.tile_pool(name="consts", bufs=1) as cpool,
            tc.tile_pool(name="xin", bufs=in_bufs) as in_pool,
            tc.tile_pool(name="yout", bufs=out_bufs) as out_pool,
            tc.tile_pool(name="scr", bufs=scr_bufs) as scr_pool,
            tc.tile_pool(name="tmp", bufs=tmp_bufs) as tmp_pool,
            tc.tile_pool(name="psum", bufs=8, space="PSUM") as ps_pool,
        ):
            cst = cpool.tile([P, 32 + P], f32)
            nc.gpsimd.dma_start(out=cst, in_=cst_d)
            w_sb = cst[:, 0:G * KTAPS]
            b_sb = cst[:, 16:16 + G]
            s_sb = cst[:, 20:20 + G * K]
            eye = cst[:, 32:32 + P]

            # per-(group, tap) diagonal weight matrices for the PE path
            dg = {}
            if any(pe_chunks):
                for g in range(G):
                    for j in range(KTAPS):
                        d = cpool.tile([P, P], f32, tag=f"diag{g}_{j}")
                        # on ACT: keeps the DVE free for the first tiles'
                        # MAC chains during the pipeline fill
                        nc.scalar.activation(
                            d, eye, mybir.ActivationFunctionType.Identity,
                            bias=0.0,
                            scale=w_sb[:, g * KTAPS + j:g * KTAPS + j + 1])
                        dg[(g, j)] = d

            def dve_chain(out_t, scr, in_t, g, col0, n, scr0, clip):
                """taps 1..3 for out cols [col0, col0+n) on DVE; tap j only
                reaches col clip-j-1 (zero past x's end; clip is huge for
                pieces whose in_t halo extends past their last column). The
                final tap writes fp16 into out_t; clipped columns finish in
                scr."""
                for j in range(1, KTAPS):
                    nj = min(n, clip - j - col0)
                    if nj <= 0:
                        continue
                    last = j == KTAPS - 1
                    nc.vector.scalar_tensor_tensor(
                        out=out_t[:, col0:col0 + nj] if last
                        else scr[:, scr0:scr0 + nj],
                        in0=in_t[:, 1 + j + col0:1 + j + col0 + nj],
                        scalar=w_sb[:, g * KTAPS + j:g * KTAPS + j + 1],
                        in1=scr[:, scr0:scr0 + nj],
                        op0=mybir.AluOpType.mult,
                        op1=mybir.AluOpType.add,
                    )
                nlast = min(n, clip - KTAPS + 1 - col0)
                if nlast < n:  # tail cols: all their taps landed in scr
                    nc.scalar.copy(out_t[:, col0 + nlast:col0 + n],
                                   scr[:, scr0 + nlast:scr0 + n])

            def pool_chain(out_t, scr, in_t, g, col0, n, scr0, clip, feed):
                """Same taps for the Pool region: the feeder engine scales
                each shifted input by w_j (DVE tensor_scalar runs 2x for
                fp32; ACT uses its free per-partition scale), Pool
                accumulates with tensor_tensor adds; the final add writes
                fp16 into out_t."""
                for j in range(1, KTAPS):
                    nj = min(n, clip - j - col0)
                    if nj <= 0:
                        continue
                    tm = tmp_pool.tile([P, nj], f32, tag=f"tmp{j}")
                    src = in_t[:, 1 + j + col0:1 + j + col0 + nj]
                    wj = w_sb[:, g * KTAPS + j:g * KTAPS + j + 1]
                    if feed == 'dve':
                        nc.vector.tensor_scalar_mul(out=tm, in0=src,
                                                    scalar1=wj)
                    else:
                        nc.scalar.activation(
                            tm, src, mybir.ActivationFunctionType.Identity,
                            bias=0.0, scale=wj)
                    last = j == KTAPS - 1
                    nc.gpsimd.tensor_tensor(
                        out=out_t[:, col0:col0 + nj] if last
                        else scr[:, scr0:scr0 + nj],
                        in0=scr[:, scr0:scr0 + nj], in1=tm,
                        op=mybir.AluOpType.add)
                nlast = min(n, clip - KTAPS + 1 - col0)
                if nlast < n:  # tail cols: all their taps landed in scr
                    nc.scalar.copy(out_t[:, col0 + nlast:col0 + n],
                                   scr[:, scr0 + nlast:scr0 + n])

            def pe_region(out_t, in_t, g, c0, nchunks, evac='act'):
                """exact fp32 matmuls: psum = sum_j diag(wj) @ in-shift.
                One PSUM tile spans the chunks (each matmul stays inside a
                bank); a single evac applies bias and rounds once to fp16
                -- on ACT (free scale+bias) or DVE (tensor_scalar, keeps
                PE's critical path off the ACT queue)."""
                for c in range(nchunks):
                    ps = ps_pool.tile([P, MMCOLS], f32)
                    base = 1 + c0 + c * MMCOLS
                    for j in range(KTAPS):
                        nc.tensor.matmul(
                            ps, dg[(g, j)],
                            in_t[:, base + j:base + j + MMCOLS],
                            start=(j == 0), stop=(j == KTAPS - 1))
                    out_sl = out_t[:, c0 + c * MMCOLS:c0 + (c + 1) * MMCOLS]
                    if evac == 'act':
                        nc.scalar.activation(
                            out_sl, ps,
                            mybir.ActivationFunctionType.Identity,
                            bias=b_sb[:, g:g + 1], scale=1.0)
                    else:
                        nc.vector.tensor_scalar(
                            out=out_sl, in0=ps,
                            scalar1=b_sb[:, g:g + 1], scalar2=None,
                            op0=mybir.AluOpType.add)

            def tap0(scr, in_t, g, col0, n, scr0, eng='dve'):
                """scr = w0 * x_shift + bias. On DVE, tensor_scalar with two
                per-partition scalars runs at 2x for fp32; on ACT it's the
                free scale+bias of an Identity activation."""
                if eng == 'dve':
                    nc.vector.tensor_scalar(
                        out=scr[:, scr0:scr0 + n],
                        in0=in_t[:, 1 + col0:1 + col0 + n],
                        scalar1=w_sb[:, g * KTAPS:g * KTAPS + 1],
                        scalar2=b_sb[:, g:g + 1],
                        op0=mybir.AluOpType.mult,
                        op1=mybir.AluOpType.add)
                else:
                    nc.scalar.activation(
                        scr[:, scr0:scr0 + n], in_t[:, 1 + col0:1 + col0 + n],
                        mybir.ActivationFunctionType.Identity,
                        bias=b_sb[:, g:g + 1],
                        scale=w_sb[:, g * KTAPS:g * KTAPS + 1])

            W = L // col_pieces
            # out-DMAs can be emitted `out_lag` tiles late on the Pool SEQ:
            # early on, the exclusive DMA engines then serve in-DMAs
            # back-to-back instead of interleaving ready outs, so every
            # compute chain starts earlier and the held-back outs fill the
            # drain tail. The waits are long-satisfied at emission, so the
            # deferred gens never stall the Pool sequencer.
            pending_outs = []

            def flush_out():
                for args in pending_outs.pop(0):
                    nc.gpsimd.dma_start(**args)

            for t in range(NTILES):
                g = t % G  # channel group (tile order: batch-major)
                rows = slice(t * P, (t + 1) * P)
                if out_lag and t >= out_lag and pending_outs:
                    flush_out()
                for h in range(col_pieces):
                    final = h == col_pieces - 1   # piece with the zero tail
                    last = final and t == NTILES - 1
                    loutp = W + (K if final else 0)
                    # in_t: col 0 pad (16B align), then K+W xp columns.
                    # Piece 0's K-prefix is the learnable state (ACT copy);
                    # later pieces read it from x along with their W cols
                    # (the in-DMA overlaps the previous piece by K cols).
                    in_t = in_pool.tile([P, 1 + K + W], f32, tag="x")
                    if h == 0:
                        nc.sync.dma_start(out=in_t[:, 1 + K:],
                                          in_=xs[rows, :W])
                        nc.scalar.copy(in_t[:, 1:1 + K],
                                       s_sb[:, g * K:(g + 1) * K])
                    else:
                        nc.sync.dma_start(
                            out=in_t[:, 1:],
                            in_=xs[rows, h * W - K:(h + 1) * W])

                    out_t = out_pool.tile([P, loutp], f16, tag=f"out{h}")
                    npe = lt_pe if (last and lt_pe is not None) \
                        else pe_chunks[t * col_pieces + h]
                    ncols = npe * MMCOLS        # PE-covered prefix
                    nd = loutp - ncols          # tap0 + DVE/Pool suffix
                    dc = dve_cols[t * col_pieces + h] \
                        if isinstance(dve_cols, tuple) else dve_cols
                    if last and lt_dve is not None:
                        dc = lt_dve
                    dn = min(dc, nd)            # DVE subregion
                    pn = nd - dn                # Pool subregion
                    clip = loutp if final else 10 * LOUT
                    if pe_wait:
                        # pace matmuls into back-to-back stretches: the PE
                        # p-state model only reaches full clock after 3us of
                        # continuous execution, so scattered per-tile
                        # stretches pay a mid-clock intro every tile
                        with tc.tile_wait_until(
                                (pe_wait[0] + pe_wait[1] * t) / 1000.0):
                            pe_region(out_t, in_t, g, 0, npe, evac)
                    else:
                        pe_region(out_t, in_t, g, 0, npe, evac)
                    # separate d/p scratches: each chain's buffer rotation
                    # then only waits on its own region two tiles back, not
                    # on the other engine's chain as well
                    scr_d = scr_pool.tile([P, max(dn, 1)], f32,
                                          tag=f"scrd{h}")
                    scr_p = scr_pool.tile([P, max(pn, 1)], f32,
                                          tag=f"scrp{h}")
                    # Pool region's tap0 is issued first: on DVE it must not
                    # queue behind the whole DVE MAC chain, or the Pool
                    # chain (and the out-DMA) start a chain-latency late.
                    if pn > 0 and p_tap0_first:
                        tap0(scr_p, in_t, g, ncols + dn, pn, 0, tap0_p)
                    # optionally split each MAC chain into sub-chains: same
                    # work, ~1/nsub the latency before the out can fire
                    for (cn, base, width) in (("d", 0, dn), ("p", dn, pn)):
                        nsub = chain_splits if width >= 2 * chain_splits \
                            else 1
                        sub = (width + nsub - 1) // nsub
                        for s in range(nsub):
                            o = base + s * sub
                            n = min(sub, width - s * sub)
                            if n <= 0:
                                continue
                            if cn == "d":
                                if tap0_d_hi:
                                    # tap0_d gates the whole DVE MAC chain;
                                    # let it jump the ACT ready-queue ahead
                                    # of the Pool feeds
                                    with tc.high_priority():
                                        tap0(scr_d, in_t, g, ncols + o, n, o,
                                             tap0_d)
                                else:
                                    tap0(scr_d, in_t, g, ncols + o, n, o,
                                         tap0_d)
                                dve_chain(out_t, scr_d, in_t, g, ncols + o,
                                          n, o, clip)
                            else:
                                if not p_tap0_first:
                                    tap0(scr_p, in_t, g, ncols + o, n,
                                         o - dn, tap0_p)
                                pool_chain(out_t, scr_p, in_t, g, ncols + o,
                                           n, o - dn, clip, feed)

                    # out-DMAs ride the Pool SWDGE ring: waits stall only
                    # the Pool sequencer; both HWDGE rings stay wait-free.
                    osplit = final and t >= NTILES - out_split_tiles
                    ob = (0,) + tuple(
                        c for c in (out_cols if osplit else ()) if c < loutp
                    ) + (loutp,)
                    specs = [
                        dict(out=out_d[rows, h * W + b0:h * W + b1],
                             in_=out_t[:, b0:b1])
                        for b0, b1 in zip(ob[:-1], ob[1:])]
                    if out_lag == 0:
                        if out_wait and t < (out_wait[2] if
                                             len(out_wait) > 2 else NTILES):
                            # hold tile t's out until ~us = a + b*t so the
                            # DMA engines serve in-DMAs first early on
                            a_us, b_us = out_wait[:2]
                            with tc.tile_wait_until(
                                    (a_us + b_us * t) / 1000.0):
                                for a in specs:
                                    nc.gpsimd.dma_start(**a)
                        else:
                            for a in specs:
                                nc.gpsimd.dma_start(**a)
                    elif h == 0:
                        pending_outs.append(specs)
                    else:
                        pending_outs[-1].extend(specs)
            while pending_outs:
                flush_out()

    nc.compile()
    return nc


def kernel(x, weight, bias, init_state):
    from concourse.bass_utils import run_bass_kernel_spmd

    assert x.shape == (B, D, L) and x.dtype == np.float32
    wl = np.ascontiguousarray(weight[:, 0, :], dtype=np.float32)      # [D, 4]
    bias = np.ascontiguousarray(bias, dtype=np.float32)               # [D]
    st = np.ascontiguousarray(init_state, dtype=np.float32)           # [D, 3]

    if "nc" not in _CACHE:
        _CACHE["nc"] = _build_program()
    nc = _CACHE["nc"]

    in_maps = []
    for c in range(NCORES):
        lo, hi = c * DSH, (c + 1) * DSH
        xs = np.ascontiguousarray(x[:, lo:hi, :]).reshape(ROWS, L)
        wc = wl[lo:hi]                                                # [512, 4]
        prm = np.zeros((P, 32), np.float32)
        prm[:, 0:G * KTAPS] = (
            wc.reshape(G, P, KTAPS).transpose(1, 0, 2).reshape(P, G * KTAPS))
        prm[:, 16:16 + G] = bias[lo:hi].reshape(G, P).T
        prm[:, 20:20 + G * K] = (
            st[lo:hi].reshape(G, P, K).transpose(1, 0, 2).reshape(P, G * K))
        cst = np.concatenate([prm, np.eye(P, dtype=np.float32)], axis=1)
        in_maps.append({"xs": xs, "cst": cst})

    res = run_bass_kernel_spmd(nc, in_maps, core_ids=list(range(NCORES)))
    shards = [r["out"].reshape(B, DSH, LOUT) for r in res.results]
    return np.concatenate(shards, axis=1).astype(np.float32)


# revision 82
# speedup vs baseline: 1.0609x; 1.0609x over previous
"""Causal depthwise conv1d with learnable hidden-state prefix, on 8 TRN2 cores.

Reference computation (per batch b, channel d):
    xp = concat([init_state[d, :3], x[b, d, :]])          # [L+3] = [4099]
    out[b, d, t] = bias[d] + sum_{j=0..3} w[d, j] * xp[t+j]   for t in [0, 4099)
    (xp index beyond 4098 contributes 0)

Sharding: channel dim D=4096 split 8 ways (512 channels/core), zero
communication. Each core processes rows (b, d_local) = 4*512 = 2048 rows of
length 4096 -> 16 SBUF tiles of [128 rows, full row].

I/O strategy: x streams in as fp32 (exact), the result streams out as fp16
(one final rounding of each output value: rel err ~2^-11, far inside the
tolerance) and is upcast to fp32 on the host. That puts the per-core DMA
floor at 32MB in + 16MB out ~= 140us @ 360GB/s.

Compute: all engines share the row so each stays under the DMA floor. Per
tile the columns split three ways:
  - PE chunks (exact fp32 matmuls, diagonal weight per tap, 4 taps
    accumulated in PSUM); ACT evacuates + bias -> fp16.  (float32r would be
    4x cheaper but is bf16-rounded on hardware -- fails the small-|y|
    relative-error floor.)
  - DVE region: ACT does tap0+bias into an fp32 scratch, DVE runs fused
    scalar*tensor+tensor MACs for taps 1-2, tap 3 writes fp16 directly.
  - Pool region: GPSIMD can't run scalar_tensor_tensor (walrus ISA check),
    so its tap0 is a DVE tensor_scalar (2x mode for fp32), ACT forms each
    tap product via its free per-partition scale (tmp = w_j * x_shift),
    and Pool does plain tensor_tensor adds; the last add writes fp16.

Scheduling: tile_wait_until clock-waits hold each tile's out-DMA to
~(25 + 5t) us and pace the matmuls to ~(25 + 6.5t) us, so the exclusive
DMA engines stream all in-DMAs back-to-back first and the held outs fill
the drain tail; the last two tiles' outs split at col 1024 and lean on
the PE (it drains ~15us before the other engines).

Engine budget per core (TimelineSim): DMA 140.1us (the floor), PE ~122,
Pool ~120, DVE ~115, ACT ~112 -> 145,121 ns vs the 139,833 ns pure-DMA
bound; rel err 5.0e-4 on device (gate 2e-2).
"""

import numpy as np

B, D, L = 4, 4096, 4096
KTAPS = 4
K = KTAPS - 1          # 3: state length
LOUT = L + K           # 4099
NCORES = 8
DSH = D // NCORES      # 512 channels per core
ROWS = B * DSH         # 2048 rows per core
P = 128                # SBUF partitions
NTILES = ROWS // P     # 16
G = DSH // P           # 4 channel groups per core

_CACHE = {}

MMCOLS = 512           # one PSUM bank of fp32 per matmul
# fp32 matmul chunks per tile; the last two tiles lean on the PE (it drains
# ~15us before the other engines, and a bigger PE share shortens the final
# MAC chains that gate the drain tail)
PE_CHUNKS = (2,) * (NTILES - 2) + (3, 3)
DVE_COLS = 2048            # DVE-region width; Pool gets the remainder


def _build_program(pe_chunks=PE_CHUNKS, dve_cols=DVE_COLS, in_bufs=6,
                   out_bufs=5, scr_bufs=2, tmp_bufs=2, feed='act',
                   split_in=(), chain_splits=1, col_pieces=1, lt_pe=None,
                   lt_dve=None, out_cols=(1024,), tap0_d='act',
                   tap0_p='dve', evac='act', p_tap0_first=True,
                   out_split_tiles=2, out_lag=0, out_wait=(25, 5.0),
                   pe_wait=(25, 6.5), tap0_d_hi=False):
    import concourse.bacc as bacc
    import concourse.mybir as mybir
    from concourse.tile import TileContext

    f32 = mybir.dt.float32
    f16 = mybir.dt.float16
    nc = bacc.Bacc("TRN2", target_bir_lowering=False, debug=False)

    xs = nc.dram_tensor("xs", [ROWS, L], f32, kind="ExternalInput").ap()
    # single packed param tensor -> single DMA -> single sync wait downstream.
    # layout per partition p: cols [g*4+j]=w[g*128+p, j] for g<4,j<4 (0..16),
    # col 16+g = bias[g*128+p], col 20+g*3+k = init_state[g*128+p, k]
    # prm and eye ride ONE dram tensor/DMA: a second HWDGE generation on
    # the SP ring would delay the first x tile's transfer by ~625ns
    cst_d = nc.dram_tensor("cst", [P, 32 + P], f32, kind="ExternalInput").ap()
    out_d = nc.dram_tensor("out", [ROWS, LOUT], f16, kind="ExternalOutput").ap()

    with TileContext(nc) as tc:
        with (
            tc.tile_pool(name="consts", bufs=1) as cpool,
            tc.tile_pool(name="xin", bufs=in_bufs) as in_pool,
            tc.tile_pool(name="yout", bufs=out_bufs) as out_pool,
            tc.tile_pool(name="scr", bufs=scr_bufs) as scr_pool,
            tc.tile_pool(name="tmp", bufs=tmp_bufs) as tmp_pool,
            tc.tile_pool(name="psum", bufs=8, space="PSUM") as ps_pool,
        ):
            cst = cpool.tile([P, 32 + P], f32)
            nc.sync.dma_start(out=cst, in_=cst_d)
            w_sb = cst[:, 0:G * KTAPS]
            b_sb = cst[:, 16:16 + G]
            s_sb = cst[:, 20:20 + G * K]
            eye = cst[:, 32:32 + P]

            # per-(group, tap) diagonal weight matrices for the PE path
            dg = {}
            if any(pe_chunks):
                for g in range(G):
                    for j in range(KTAPS):
                        d = cpool.tile([P, P], f32, tag=f"diag{g}_{j}")
                        # on ACT: keeps the DVE free for the first tiles'
                        # MAC chains during the pipeline fill
                        nc.scalar.activation(
                            d, eye, mybir.ActivationFunctionType.Identity,
                            bias=0.0,
                            scale=w_sb[:, g * KTAPS + j:g * KTAPS + j + 1])
                        dg[(g, j)] = d

            def dve_chain(out_t, scr, in_t, g, col0, n, scr0, clip):
                """taps 1..3 for out cols [col0, col0+n) on DVE; tap j only
                reaches col clip-j-1 (zero past x's end; clip is huge for
                pieces whose in_t halo extends past their last column). The
                final tap writes fp16 into out_t; clipped columns finish in
                scr."""
                for j in range(1, KTAPS):
                    nj = min(n, clip - j - col0)
                    if nj <= 0:
                        continue
                    last = j == KTAPS - 1
                    nc.vector.scalar_tensor_tensor(
                        out=out_t[:, col0:col0 + nj] if last
                        else scr[:, scr0:scr0 + nj],
                        in0=in_t[:, 1 + j + col0:1 + j + col0 + nj],
                        scalar=w_sb[:, g * KTAPS + j:g * KTAPS + j + 1],
                        in1=scr[:, scr0:scr0 + nj],
                        op0=mybir.AluOpType.mult,
                        op1=mybir.AluOpType.add,
                    )
                nlast = min(n, clip - KTAPS + 1 - col0)
                if nlast < n:  # tail cols: all their taps landed in scr
                    nc.scalar.copy(out_t[:, col0 + nlast:col0 + n],
                                   scr[:, scr0 + nlast:scr0 + n])

            def pool_chain(out_t, scr, in_t, g, col0, n, scr0, clip, feed):
                """Same taps for the Pool region: the feeder engine scales
                each shifted input by w_j (DVE tensor_scalar runs 2x for
                fp32; ACT uses its free per-partition scale), Pool
                accumulates with tensor_tensor adds; the final add writes
                fp16 into out_t."""
                for j in range(1, KTAPS):
                    nj = min(n, clip - j - col0)
                    if nj <= 0:
                        continue
                    tm = tmp_pool.tile([P, nj], f32, tag=f"tmp{j}")
                    src = in_t[:, 1 + j + col0:1 + j + col0 + nj]
                    wj = w_sb[:, g * KTAPS + j:g * KTAPS + j + 1]
                    if feed == 'dve':
                        nc.vector.tensor_scalar_mul(out=tm, in0=src,
                                                    scalar1=wj)
                    else:
                        nc.scalar.activation(
                            tm, src, mybir.ActivationFunctionType.Identity,
                            bias=0.0, scale=wj)
                    last = j == KTAPS - 1
                    nc.gpsimd.tensor_tensor(
                        out=out_t[:, col0:col0 + nj] if last
                        else scr[:, scr0:scr0 + nj],
                        in0=scr[:, scr0:scr0 + nj], in1=tm,
                        op=mybir.AluOpType.add)
                nlast = min(n, clip - KTAPS + 1 - col0)
                if nlast < n:  # tail cols: all their taps landed in scr
                    nc.scalar.copy(out_t[:, col0 + nlast:col0 + n],
                                   scr[:, scr0 + nlast:scr0 + n])

            def pe_region(out_t, in_t, g, c0, nchunks, evac='act'):
                """exact fp32 matmuls: psum = sum_j diag(wj) @ in-shift.
                One PSUM tile spans the chunks (each matmul stays inside a
                bank); a single evac applies bias and rounds once to fp16
                -- on ACT (free scale+bias) or DVE (tensor_scalar, keeps
                PE's critical path off the ACT queue)."""
                for c in range(nchunks):
                    ps = ps_pool.tile([P, MMCOLS], f32)
                    base = 1 + c0 + c * MMCOLS
                    for j in range(KTAPS):
                        nc.tensor.matmul(
                            ps, dg[(g, j)],
                            in_t[:, base + j:base + j + MMCOLS],
                            start=(j == 0), stop=(j == KTAPS - 1))
                    out_sl = out_t[:, c0 + c * MMCOLS:c0 + (c + 1) * MMCOLS]
                    if evac == 'act':
                        nc.scalar.activation(
                            out_sl, ps,
                            mybir.ActivationFunctionType.Identity,
                            bias=b_sb[:, g:g + 1], scale=1.0)
                    else:
                        nc.vector.tensor_scalar(
                            out=out_sl, in0=ps,
                            scalar1=b_sb[:, g:g + 1], scalar2=None,
                            op0=mybir.AluOpType.add)

            def tap0(scr, in_t, g, col0, n, scr0, eng='dve'):
                """scr = w0 * x_shift + bias. On DVE, tensor_scalar with two
                per-partition scalars runs at 2x for fp32; on ACT it's the
                free scale+bias of an Identity activation."""
                if eng == 'dve':
                    nc.vector.tensor_scalar(
                        out=scr[:, scr0:scr0 + n],
                        in0=in_t[:, 1 + col0:1 + col0 + n],
                        scalar1=w_sb[:, g * KTAPS:g * KTAPS + 1],
                        scalar2=b_sb[:, g:g + 1],
                        op0=mybir.AluOpType.mult,
                        op1=mybir.AluOpType.add)
                else:
                    nc.scalar.activation(
                        scr[:, scr0:scr0 + n], in_t[:, 1 + col0:1 + col0 + n],
                        mybir.ActivationFunctionType.Identity,
                        bias=b_sb[:, g:g + 1],
                        scale=w_sb[:, g * KTAPS:g * KTAPS + 1])

            W = L // col_pieces
            # out-DMAs can be emitted `out_lag` tiles late on the Pool SEQ:
            # early on, the exclusive DMA engines then serve in-DMAs
            # back-to-back instead of interleaving ready outs, so every
            # compute chain starts earlier and the held-back outs fill the
            # drain tail. The waits are long-satisfied at emission, so the
            # deferred gens never stall the Pool sequencer.
            pending_outs = []

            def flush_out():
                for args in pending_outs.pop(0):
                    nc.gpsimd.dma_start(**args)

            for t in range(NTILES):
                g = t % G  # channel group (tile order: batch-major)
                rows = slice(t * P, (t + 1) * P)
                if out_lag and t >= out_lag and pending_outs:
                    flush_out()
                for h in range(col_pieces):
                    final = h == col_pieces - 1   # piece with the zero tail
                    last = final and t == NTILES - 1
                    loutp = W + (K if final else 0)
                    # in_t: col 0 pad (16B align), then K+W xp columns.
                    # Piece 0's K-prefix is the learnable state (ACT copy);
                    # later pieces read it from x along with their W cols
                    # (the in-DMA overlaps the previous piece by K cols).
                    in_t = in_pool.tile([P, 1 + K + W], f32, tag="x")
                    if h == 0:
                        nc.sync.dma_start(out=in_t[:, 1 + K:],
                                          in_=xs[rows, :W])
                        nc.scalar.copy(in_t[:, 1:1 + K],
                                       s_sb[:, g * K:(g + 1) * K])
                    else:
                        nc.sync.dma_start(
                            out=in_t[:, 1:],
                            in_=xs[rows, h * W - K:(h + 1) * W])

                    out_t = out_pool.tile([P, loutp], f16, tag=f"out{h}")
                    npe = lt_pe if (last and lt_pe is not None) \
                        else pe_chunks[t * col_pieces + h]
                    ncols = npe * MMCOLS        # PE-covered prefix
                    nd = loutp - ncols          # tap0 + DVE/Pool suffix
                    dc = dve_cols[t * col_pieces + h] \
                        if isinstance(dve_cols, tuple) else dve_cols
                    if last and lt_dve is not None:
                        dc = lt_dve
                    dn = min(dc, nd)            # DVE subregion
                    pn = nd - dn                # Pool subregion
                    clip = loutp if final else 10 * LOUT
                    if pe_wait:
                        # pace matmuls into back-to-back stretches: the PE
                        # p-state model only reaches full clock after 3us of
                        # continuous execution, so scattered per-tile
                        # stretches pay a mid-clock intro every tile
                        with tc.tile_wait_until(
                                (pe_wait[0] + pe_wait[1] * t) / 1000.0):
                            pe_region(out_t, in_t, g, 0, npe, evac)
                    else:
                        pe_region(out_t, in_t, g, 0, npe, evac)
                    # separate d/p scratches: each chain's buffer rotation
                    # then only waits on its own region two tiles back, not
                    # on the other engine's chain as well
                    scr_d = scr_pool.tile([P, max(dn, 1)], f32,
                                          tag=f"scrd{h}")
                    scr_p = scr_pool.tile([P, max(pn, 1)], f32,
                                          tag=f"scrp{h}")
                    # Pool region's tap0 is issued first: on DVE it must not
                    # queue behind the whole DVE MAC chain, or the Pool
                    # chain (and the out-DMA) start a chain-latency late.
                    if pn > 0 and p_tap0_first:
                        tap0(scr_p, in_t, g, ncols + dn, pn, 0, tap0_p)
                    # optionally split each MAC chain into sub-chains: same
                    # work, ~1/nsub the latency before the out can fire
                    for (cn, base, width) in (("d", 0, dn), ("p", dn, pn)):
                        nsub = chain_splits if width >= 2 * chain_splits \
                            else 1
                        sub = (width + nsub - 1) // nsub
                        for s in range(nsub):
                            o = base + s * sub
                            n = min(sub, width - s * sub)
                            if n <= 0:
                                continue
                            if cn == "d":
                                if tap0_d_hi:
                                    # tap0_d gates the whole DVE MAC chain;
                                    # let it jump the ACT ready-queue ahead
                                    # of the Pool feeds
                                    with tc.high_priority():
                                        tap0(scr_d, in_t, g, ncols + o, n, o,
                                             tap0_d)
                                else:
                                    tap0(scr_d, in_t, g, ncols + o, n, o,
                                         tap0_d)
                                dve_chain(out_t, scr_d, in_t, g, ncols + o,
                                          n, o, clip)
                            else:
                                if not p_tap0_first:
                                    tap0(scr_p, in_t, g, ncols + o, n,
                                         o - dn, tap0_p)
                                pool_chain(out_t, scr_p, in_t, g, ncols + o,
                                           n, o - dn, clip, feed)

                    # out-DMAs ride the Pool SWDGE ring: waits stall only
                    # the Pool sequencer; both HWDGE rings stay wait-free.
                    osplit = final and t >= NTILES - out_split_tiles
                    ob = (0,) + tuple(
                        c for c in (out_cols if osplit else ()) if c < loutp
                    ) + (loutp,)
                    specs = [
                        dict(out=out_d[rows, h * W + b0:h * W + b1],
                             in_=out_t[:, b0:b1])
                        for b0, b1 in zip(ob[:-1], ob[1:])]
                    if out_lag == 0:
                        if out_wait and t < (out_wait[2] if
                                             len(out_wait) > 2 else NTILES):
                            # hold tile t's out until ~us = a + b*t so the
                            # DMA engines serve in-DMAs first early on
                            a_us, b_us = out_wait[:2]
                            with tc.tile_wait_until(
                                    (a_us + b_us * t) / 1000.0):
                                for a in specs:
                                    nc.gpsimd.dma_start(**a)
                        else:
                            for a in specs:
                                nc.gpsimd.dma_start(**a)
                    elif h == 0:
                        pending_outs.append(specs)
                    else:
                        pending_outs[-1].extend(specs)
            while pending_outs:
                flush_out()

    nc.compile()
    return nc


def kernel(x, weight, bias, init_state):
    from concourse.bass_utils import run_bass_kernel_spmd

    assert x.shape == (B, D, L) and x.dtype == np.float32
    wl = np.ascontiguousarray(weight[:, 0, :], dtype=np.float32)      # [D, 4]
    bias = np.ascontiguousarray(bias, dtype=np.float32)               # [D]
    st = np.ascontiguousarray(init_state, dtype=np.float32)           # [D, 3]

    if "nc" not in _CACHE:
        _CACHE["nc"] = _build_program()
    nc = _CACHE["nc"]

    in_maps = []
    for c in range(NCORES):
        lo, hi = c * DSH, (c + 1) * DSH
        xs = np.ascontiguousarray(x[:, lo:hi, :]).reshape(ROWS, L)
        wc = wl[lo:hi]                                                # [512, 4]
        prm = np.zeros((P, 32), np.float32)
        prm[:, 0:G * KTAPS] = (
            wc.reshape(G, P, KTAPS).transpose(1, 0, 2).reshape(P, G * KTAPS))
        prm[:, 16:16 + G] = bias[lo:hi].reshape(G, P).T
        prm[:, 20:20 + G * K] = (
            st[lo:hi].reshape(G, P, K).transpose(1, 0, 2).reshape(P, G * K))
        cst = np.concatenate([prm, np.eye(P, dtype=np.float32)], axis=1)
        in_maps.append({"xs": xs, "cst": cst})

    res = run_bass_kernel_spmd(nc, in_maps, core_ids=list(range(NCORES)))
    shards = [r["out"].reshape(B, DSH, LOUT) for r in res.results]
    return np.concatenate(shards, axis=1).astype(np.float32)


# revision 95
# speedup vs baseline: 1.0663x; 1.0050x over previous
"""Causal depthwise conv1d with learnable hidden-state prefix, on 8 TRN2 cores.

Reference computation (per batch b, channel d):
    xp = concat([init_state[d, :3], x[b, d, :]])          # [L+3] = [4099]
    out[b, d, t] = bias[d] + sum_{j=0..3} w[d, j] * xp[t+j]   for t in [0, 4099)
    (xp index beyond 4098 contributes 0)

Sharding: channel dim D=4096 split 8 ways (512 channels/core), zero
communication. Each core processes rows (b, d_local) = 4*512 = 2048 rows of
length 4096 -> 16 SBUF tiles of [128 rows, full row].

I/O strategy: x streams in as fp32 (exact), the result streams out as fp16
(one final rounding of each output value: rel err ~2^-11, far inside the
tolerance) and is upcast to fp32 on the host. That puts the per-core DMA
floor at 32MB in + 16MB out ~= 140us @ 360GB/s.

Compute: all engines share the row so each stays under the DMA floor. Per
tile the columns split three ways:
  - PE chunks (exact fp32 matmuls, diagonal weight per tap, 4 taps
    accumulated in PSUM); ACT evacuates + bias -> fp16.  (float32r would be
    4x cheaper but is bf16-rounded on hardware -- fails the small-|y|
    relative-error floor.)
  - DVE region: ACT does tap0+bias into an fp32 scratch, DVE runs fused
    scalar*tensor+tensor MACs for taps 1-2, tap 3 writes fp16 directly.
  - Pool region: GPSIMD can't run scalar_tensor_tensor (walrus ISA check),
    so its tap0 is a DVE tensor_scalar (2x mode for fp32), ACT forms each
    tap product via its free per-partition scale (tmp = w_j * x_shift),
    and Pool does plain tensor_tensor adds; the last add writes fp16.

Scheduling: tile_wait_until clock-waits hold each tile's out-DMA to
~(25 + 5t) us and pace the matmuls to ~(25 + 6.5t) us, so the exclusive
DMA engines stream all in-DMAs back-to-back first and the held outs fill
the drain tail. The last tiles lean on the PE (3 matmul chunks; it drains
~15us before the other engines), split their outs at col 1024, and the
last four tiles' outs ride the idle SP HWDGE ring (gen 625ns vs the Pool
SWDGE's 1038ns -- that generation sits serially between the final MAC op
and the last transfer).

Engine budget per core (TimelineSim): DMA 140.1us (the floor), PE ~129,
Pool ~115, DVE ~114, ACT ~110 -> 143,900 ns = 1.97us head + 140.1 busy +
~0.3 bubbles + ~1.5 final semaphore/drain, vs the 139,833 ns pure-DMA
bound; rel err 5.0e-4 on device (gate 2e-2).
"""

import numpy as np

B, D, L = 4, 4096, 4096
KTAPS = 4
K = KTAPS - 1          # 3: state length
LOUT = L + K           # 4099
NCORES = 8
DSH = D // NCORES      # 512 channels per core
ROWS = B * DSH         # 2048 rows per core
P = 128                # SBUF partitions
NTILES = ROWS // P     # 16
G = DSH // P           # 4 channel groups per core

_CACHE = {}

MMCOLS = 512           # one PSUM bank of fp32 per matmul
# fp32 matmul chunks per tile; the last two tiles lean on the PE (it drains
# ~15us before the other engines, and a bigger PE share shortens the final
# MAC chains that gate the drain tail)
PE_CHUNKS = (2,) * (NTILES - 2) + (3, 3)
DVE_COLS = 2048            # DVE-region width; Pool gets the remainder


def _build_program(pe_chunks=PE_CHUNKS, dve_cols=DVE_COLS, in_bufs=6,
                   out_bufs=6, scr_bufs=2, tmp_bufs=2, feed='act',
                   split_in=(), chain_splits=1, col_pieces=1, lt_pe=None,
                   lt_dve=None, out_cols=(1024,), tap0_d='act',
                   tap0_p='dve', evac='act', p_tap0_first=True,
                   out_split_tiles=3, out_lag=0, out_wait=(25, 5.0),
                   pe_wait=(25, 6.5), tap0_d_hi=False, lt_out_sp=True):
    import concourse.bacc as bacc
    import concourse.mybir as mybir
    from concourse.tile import TileContext

    f32 = mybir.dt.float32
    f16 = mybir.dt.float16
    nc = bacc.Bacc("TRN2", target_bir_lowering=False, debug=False)

    xs = nc.dram_tensor("xs", [ROWS, L], f32, kind="ExternalInput").ap()
    # single packed param tensor -> single DMA -> single sync wait downstream.
    # layout per partition p: cols [g*4+j]=w[g*128+p, j] for g<4,j<4 (0..16),
    # col 16+g = bias[g*128+p], col 20+g*3+k = init_state[g*128+p, k]
    # prm and eye ride ONE dram tensor/DMA: a second HWDGE generation on
    # the SP ring would delay the first x tile's transfer by ~625ns
    cst_d = nc.dram_tensor("cst", [P, 32 + P], f32, kind="ExternalInput").ap()
    out_d = nc.dram_tensor("out", [ROWS, LOUT], f16, kind="ExternalOutput").ap()

    with TileContext(nc) as tc:
        with (
            tc.tile_pool(name="consts", bufs=1) as cpool,
            tc.tile_pool(name="xin", bufs=in_bufs) as in_pool,
            tc.tile_pool(name="yout", bufs=out_bufs) as out_pool,
            tc.tile_pool(name="scr", bufs=scr_bufs) as scr_pool,
            tc.tile_pool(name="tmp", bufs=tmp_bufs) as tmp_pool,
            tc.tile_pool(name="psum", bufs=8, space="PSUM") as ps_pool,
        ):
            cst = cpool.tile([P, 32 + P], f32)
            nc.scalar.dma_start(out=cst, in_=cst_d)
            w_sb = cst[:, 0:G * KTAPS]
            b_sb = cst[:, 16:16 + G]
            s_sb = cst[:, 20:20 + G * K]
            eye = cst[:, 32:32 + P]

            # per-(group, tap) diagonal weight matrices for the PE path
            dg = {}
            if any(pe_chunks):
                for g in range(G):
                    for j in range(KTAPS):
                        d = cpool.tile([P, P], f32, tag=f"diag{g}_{j}")
                        # on ACT: keeps the DVE free for the first tiles'
                        # MAC chains during the pipeline fill
                        nc.scalar.activation(
                            d, eye, mybir.ActivationFunctionType.Identity,
                            bias=0.0,
                            scale=w_sb[:, g * KTAPS + j:g * KTAPS + j + 1])
                        dg[(g, j)] = d

            def dve_chain(out_t, scr, in_t, g, col0, n, scr0, clip):
                """taps 1..3 for out cols [col0, col0+n) on DVE; tap j only
                reaches col clip-j-1 (zero past x's end; clip is huge for
                pieces whose in_t halo extends past their last column). The
                final tap writes fp16 into out_t; clipped columns finish in
                scr."""
                for j in range(1, KTAPS):
                    nj = min(n, clip - j - col0)
                    if nj <= 0:
                        continue
                    last = j == KTAPS - 1
                    nc.vector.scalar_tensor_tensor(
                        out=out_t[:, col0:col0 + nj] if last
                        else scr[:, scr0:scr0 + nj],
                        in0=in_t[:, 1 + j + col0:1 + j + col0 + nj],
                        scalar=w_sb[:, g * KTAPS + j:g * KTAPS + j + 1],
                        in1=scr[:, scr0:scr0 + nj],
                        op0=mybir.AluOpType.mult,
                        op1=mybir.AluOpType.add,
                    )
                nlast = min(n, clip - KTAPS + 1 - col0)
                if nlast < n:  # tail cols: all their taps landed in scr
                    nc.scalar.copy(out_t[:, col0 + nlast:col0 + n],
                                   scr[:, scr0 + nlast:scr0 + n])

            def pool_chain(out_t, scr, in_t, g, col0, n, scr0, clip, feed):
                """Same taps for the Pool region: the feeder engine scales
                each shifted input by w_j (DVE tensor_scalar runs 2x for
                fp32; ACT uses its free per-partition scale), Pool
                accumulates with tensor_tensor adds; the final add writes
                fp16 into out_t."""
                for j in range(1, KTAPS):
                    nj = min(n, clip - j - col0)
                    if nj <= 0:
                        continue
                    tm = tmp_pool.tile([P, nj], f32, tag=f"tmp{j}")
                    src = in_t[:, 1 + j + col0:1 + j + col0 + nj]
                    wj = w_sb[:, g * KTAPS + j:g * KTAPS + j + 1]
                    if feed == 'dve':
                        nc.vector.tensor_scalar_mul(out=tm, in0=src,
                                                    scalar1=wj)
                    else:
                        nc.scalar.activation(
                            tm, src, mybir.ActivationFunctionType.Identity,
                            bias=0.0, scale=wj)
                    last = j == KTAPS - 1
                    nc.gpsimd.tensor_tensor(
                        out=out_t[:, col0:col0 + nj] if last
                        else scr[:, scr0:scr0 + nj],
                        in0=scr[:, scr0:scr0 + nj], in1=tm,
                        op=mybir.AluOpType.add)
                nlast = min(n, clip - KTAPS + 1 - col0)
                if nlast < n:  # tail cols: all their taps landed in scr
                    nc.scalar.copy(out_t[:, col0 + nlast:col0 + n],
                                   scr[:, scr0 + nlast:scr0 + n])

            def pe_region(out_t, in_t, g, c0, nchunks, evac='act'):
                """exact fp32 matmuls: psum = sum_j diag(wj) @ in-shift.
                One PSUM tile spans the chunks (each matmul stays inside a
                bank); a single evac applies bias and rounds once to fp16
                -- on ACT (free scale+bias) or DVE (tensor_scalar, keeps
                PE's critical path off the ACT queue)."""
                for c in range(nchunks):
                    ps = ps_pool.tile([P, MMCOLS], f32)
                    base = 1 + c0 + c * MMCOLS
                    for j in range(KTAPS):
                        nc.tensor.matmul(
                            ps, dg[(g, j)],
                            in_t[:, base + j:base + j + MMCOLS],
                            start=(j == 0), stop=(j == KTAPS - 1))
                    out_sl = out_t[:, c0 + c * MMCOLS:c0 + (c + 1) * MMCOLS]
                    if evac == 'act':
                        nc.scalar.activation(
                            out_sl, ps,
                            mybir.ActivationFunctionType.Identity,
                            bias=b_sb[:, g:g + 1], scale=1.0)
                    else:
                        nc.vector.tensor_scalar(
                            out=out_sl, in0=ps,
                            scalar1=b_sb[:, g:g + 1], scalar2=None,
                            op0=mybir.AluOpType.add)

            def tap0(scr, in_t, g, col0, n, scr0, eng='dve'):
                """scr = w0 * x_shift + bias. On DVE, tensor_scalar with two
                per-partition scalars runs at 2x for fp32; on ACT it's the
                free scale+bias of an Identity activation."""
                if eng == 'dve':
                    nc.vector.tensor_scalar(
                        out=scr[:, scr0:scr0 + n],
                        in0=in_t[:, 1 + col0:1 + col0 + n],
                        scalar1=w_sb[:, g * KTAPS:g * KTAPS + 1],
                        scalar2=b_sb[:, g:g + 1],
                        op0=mybir.AluOpType.mult,
                        op1=mybir.AluOpType.add)
                else:
                    nc.scalar.activation(
                        scr[:, scr0:scr0 + n], in_t[:, 1 + col0:1 + col0 + n],
                        mybir.ActivationFunctionType.Identity,
                        bias=b_sb[:, g:g + 1],
                        scale=w_sb[:, g * KTAPS:g * KTAPS + 1])

            W = L // col_pieces
            # out-DMAs can be emitted `out_lag` tiles late on the Pool SEQ:
            # early on, the exclusive DMA engines then serve in-DMAs
            # back-to-back instead of interleaving ready outs, so every
            # compute chain starts earlier and the held-back outs fill the
            # drain tail. The waits are long-satisfied at emission, so the
            # deferred gens never stall the Pool sequencer.
            pending_outs = []

            def flush_out():
                for args in pending_outs.pop(0):
                    nc.gpsimd.dma_start(**args)

            for t in range(NTILES):
                g = t % G  # channel group (tile order: batch-major)
                rows = slice(t * P, (t + 1) * P)
                if out_lag and t >= out_lag and pending_outs:
                    flush_out()
                for h in range(col_pieces):
                    final = h == col_pieces - 1   # piece with the zero tail
                    last = final and t == NTILES - 1
                    loutp = W + (K if final else 0)
                    # in_t: col 0 pad (16B align), then K+W xp columns.
                    # Piece 0's K-prefix is the learnable state (ACT copy);
                    # later pieces read it from x along with their W cols
                    # (the in-DMA overlaps the previous piece by K cols).
                    in_t = in_pool.tile([P, 1 + K + W], f32, tag="x")
                    if h == 0:
                        nc.sync.dma_start(out=in_t[:, 1 + K:],
                                          in_=xs[rows, :W])
                        nc.scalar.copy(in_t[:, 1:1 + K],
                                       s_sb[:, g * K:(g + 1) * K])
                    else:
                        nc.sync.dma_start(
                            out=in_t[:, 1:],
                            in_=xs[rows, h * W - K:(h + 1) * W])

                    out_t = out_pool.tile([P, loutp], f16, tag=f"out{h}")
                    npe = lt_pe if (last and lt_pe is not None) \
                        else pe_chunks[t * col_pieces + h]
                    ncols = npe * MMCOLS        # PE-covered prefix
                    nd = loutp - ncols          # tap0 + DVE/Pool suffix
                    dc = dve_cols[t * col_pieces + h] \
                        if isinstance(dve_cols, tuple) else dve_cols
                    if last and lt_dve is not None:
                        dc = lt_dve
                    dn = min(dc, nd)            # DVE subregion
                    pn = nd - dn                # Pool subregion
                    clip = loutp if final else 10 * LOUT
                    if pe_wait:
                        # pace matmuls into back-to-back stretches: the PE
                        # p-state model only reaches full clock after 3us of
                        # continuous execution, so scattered per-tile
                        # stretches pay a mid-clock intro every tile
                        with tc.tile_wait_until(
                                (pe_wait[0] + pe_wait[1] * t) / 1000.0):
                            pe_region(out_t, in_t, g, 0, npe, evac)
                    else:
                        pe_region(out_t, in_t, g, 0, npe, evac)
                    # separate d/p scratches: each chain's buffer rotation
                    # then only waits on its own region two tiles back, not
                    # on the other engine's chain as well
                    scr_d = scr_pool.tile([P, max(dn, 1)], f32,
                                          tag=f"scrd{h}")
                    scr_p = scr_pool.tile([P, max(pn, 1)], f32,
                                          tag=f"scrp{h}")
                    # Pool region's tap0 is issued first: on DVE it must not
                    # queue behind the whole DVE MAC chain, or the Pool
                    # chain (and the out-DMA) start a chain-latency late.
                    if pn > 0 and p_tap0_first:
                        tap0(scr_p, in_t, g, ncols + dn, pn, 0, tap0_p)
                    # optionally split each MAC chain into sub-chains: same
                    # work, ~1/nsub the latency before the out can fire
                    for (cn, base, width) in (("d", 0, dn), ("p", dn, pn)):
                        nsub = chain_splits if width >= 2 * chain_splits \
                            else 1
                        sub = (width + nsub - 1) // nsub
                        for s in range(nsub):
                            o = base + s * sub
                            n = min(sub, width - s * sub)
                            if n <= 0:
                                continue
                            if cn == "d":
                                if tap0_d_hi:
                                    # tap0_d gates the whole DVE MAC chain;
                                    # let it jump the ACT ready-queue ahead
                                    # of the Pool feeds
                                    with tc.high_priority():
                                        tap0(scr_d, in_t, g, ncols + o, n, o,
                                             tap0_d)
                                else:
                                    tap0(scr_d, in_t, g, ncols + o, n, o,
                                         tap0_d)
                                dve_chain(out_t, scr_d, in_t, g, ncols + o,
                                          n, o, clip)
                            else:
                                if not p_tap0_first:
                                    tap0(scr_p, in_t, g, ncols + o, n,
                                         o - dn, tap0_p)
                                pool_chain(out_t, scr_p, in_t, g, ncols + o,
                                           n, o - dn, clip, feed)

                    # out-DMAs ride the Pool SWDGE ring: waits stall only
                    # the Pool sequencer; both HWDGE rings stay wait-free.
                    osplit = final and t >= NTILES - out_split_tiles
                    ob = (0,) + tuple(
                        c for c in (out_cols if osplit else ()) if c < loutp
                    ) + (loutp,)
                    specs = [
                        dict(out=out_d[rows, h * W + b0:h * W + b1],
                             in_=out_t[:, b0:b1])
                        for b0, b1 in zip(ob[:-1], ob[1:])]
                    if out_lag == 0:
                        oeng = nc.sync if (last and lt_out_sp) else nc.gpsimd
                        if out_wait and t < (out_wait[2] if
                                             len(out_wait) > 2 else NTILES):
                            # hold tile t's out until ~us = a + b*t so the
                            # DMA engines serve in-DMAs first early on
                            a_us, b_us = out_wait[:2]
                            with tc.tile_wait_until(
                                    (a_us + b_us * t) / 1000.0):
                                for a in specs:
                                    oeng.dma_start(**a)
                        else:
                            for a in specs:
                                oeng.dma_start(**a)
                    elif h == 0:
                        pending_outs.append(specs)
                    else:
                        pending_outs[-1].extend(specs)
            while pending_outs:
                flush_out()

    nc.compile()
    return nc


def kernel(x, weight, bias, init_state):
    from concourse.bass_utils import run_bass_kernel_spmd

    assert x.shape == (B, D, L) and x.dtype == np.float32
    wl = np.ascontiguousarray(weight[:, 0, :], dtype=np.float32)      # [D, 4]
    bias = np.ascontiguousarray(bias, dtype=np.float32)               # [D]
    st = np.ascontiguousarray(init_state, dtype=np.float32)           # [D, 3]

    if "nc" not in _CACHE:
        _CACHE["nc"] = _build_program()
    nc = _CACHE["nc"]

    in_maps = []
    for c in range(NCORES):
        lo, hi = c * DSH, (c + 1) * DSH
        xs = np.ascontiguousarray(x[:, lo:hi, :]).reshape(ROWS, L)
        wc = wl[lo:hi]                                                # [512, 4]
        prm = np.zeros((P, 32), np.float32)
        prm[:, 0:G * KTAPS] = (
            wc.reshape(G, P, KTAPS).transpose(1, 0, 2).reshape(P, G * KTAPS))
        prm[:, 16:16 + G] = bias[lo:hi].reshape(G, P).T
        prm[:, 20:20 + G * K] = (
            st[lo:hi].reshape(G, P, K).transpose(1, 0, 2).reshape(P, G * K))
        cst = np.concatenate([prm, np.eye(P, dtype=np.float32)], axis=1)
        in_maps.append({"xs": xs, "cst": cst})

    res = run_bass_kernel_spmd(nc, in_maps, core_ids=list(range(NCORES)))
    shards = [r["out"].reshape(B, DSH, LOUT) for r in res.results]
    return np.concatenate(shards, axis=1).astype(np.float32)
